# revision 48
# baseline (speedup 1.0000x reference)
"""Multi-head attention (B=4, S=2048, E=1024, H=16) on 8 trn2 NeuronCores.

Sharding: data-parallel over B (4) x tensor-parallel over H (2 halves of 8
heads). Core c handles batch c//2, head-half c%2. Column-parallel qkv_proj,
row-parallel out_proj; the all-reduce of the two partial outputs per batch is
done on the host during unshard (a sum of two arrays), as is the final
transpose (the device emits out^T to keep DMA writes contiguous).

Device kernel v2 (bf16 matmuls, fp32 psum), 300us vs the 403us fp32r
baseline (PE ~277us busy at 92%, ACT exp ~255us). Key structure:
  - scores^T per key-tile-triple in [128 keys, 3 jt, 512 q] psum tiles; one
    1536-wide ACT exp per tile (amortizes ACT access overhead) -> e bf16
    with the 1/sqrt(E) scale folded in.
  - PV runs TRANSPOSED: stationary = e-tile [128 keys, 128 queries], moving
    = [v_h | 1] bf16 [128, 65], psum accumulates [128 q, 64 ctx | softmax
    denominator] - halves PV cycles vs the [65, q] form and makes the
    normalizer a per-partition scalar: reciprocal + tensor_scalar_mul on
    DVE, no DRAM-bounce partition broadcast.
  - ctx [q, d] -> [d, q] via the DMA xbar (dma_start_transpose), zero PE.
  - software-pipelined emission at half-head granularity: qk-proj (JIT per
    pair) and per-pair v-proj columns are deadline-packed as fillers so the
    PE never starves while ACT paces the exps; memset-fed dummy matmuls
    keep the PE p-state hot during the initial x load (split across the
    sync+gpsimd DMA queues); pair-0 qk chunks interleave with the first
    score triples (triple g only needs k-tokens 384g..384g+384) so the
    first exp fires ~12us in; the last head runs 512-query-fine so PV /
    xbar-transpose / out-proj chase the final exps at sub-half granularity.
    All biases ride DVE evictions (qk/out: per-partition tensor_scalar_add;
    v: broadcast-DMA'd row + tensor_add), so no bias matmuls.
"""
import sys

import numpy as np

sys.path.insert(0, "/opt/trn_rl_repo")

import ml_dtypes

import concourse.bacc as bacc
import concourse.mybir as mybir
import concourse.tile as tile
from concourse.bass_utils import run_bass_kernel_spmd

F32 = mybir.dt.float32
BF16 = mybir.dt.bfloat16
EXP = mybir.ActivationFunctionType.Exp

B, S, E, H, HD = 4, 2048, 1024, 16, 64
HL = 8            # heads per core
SCALE = float(1.0 / np.sqrt(np.float32(E)))

# cons layout (bf16 row): [0:128] ones, [128:640] bv, [640:1160] v1 pad row
ONES_OFF, BV_OFF, VPAD_OFF, CONS_LEN = 0, 128, 640, 1160


def build_nc():
    nc = bacc.Bacc("TRN2", target_bir_lowering=False, debug=False, num_devices=8)
    # xw cols: [0:1024] per-pair [Wq_p^T | Wk_p^T] (4 x 256), [1024:3072] x^T,
    # [3072:3584] Wv_loc^T -- all bf16
    xw_d = nc.declare_dram_parameter("xw", [E, 3584], BF16, isOutput=False)
    bqk_d = nc.declare_dram_parameter("bqk", [E, 1], F32, isOutput=False)
    cons_d = nc.declare_dram_parameter("cons", [1, CONS_LEN], BF16, isOutput=False)
    wo_d = nc.declare_dram_parameter("wo", [512, E], BF16, isOutput=False)
    bout_d = nc.declare_dram_parameter("bout", [E, 1], F32, isOutput=False)
    out_d = nc.declare_dram_parameter("outT", [E, S], F32, isOutput=True)

    with tile.TileContext(nc) as tc:
      with tc.tile_pool(name="pp", bufs=1) as pp:
        bqk_s = pp.tile([128, 8, 1], F32)
        bout_s = pp.tile([128, 8, 1], F32)
        nc.gpsimd.dma_start(out=bqk_s, in_=bqk_d[:, :].rearrange("(m p) o -> p m o", p=128))
        # warm the ACT exp table (load is ~1.3us; keep it off the critical path)
        warm = pp.tile([1, 1], F32)
        nc.scalar.activation(out=warm, in_=bqk_s[0:1, 0, 0:1], func=EXP)

        with tc.tile_pool(name="pa", bufs=1) as pa:
            # persistent phase-3 state
            qk_s = pa.tile([128, 2, 2, S], BF16)        # [slot, q|k, tok]
            v1_s = pa.tile([128, 16, 520], BF16)        # per jt: 8 x [v_h(64) | 1]
            e_s = [pa.tile([128, 16, 1024], BF16, name=f"e{i}") for i in range(3)]
            ctq_s = pa.tile([128, 2, 16, 128], BF16)    # [slot, qb, d-pair]
            ctx_s = pa.tile([128, 4, S], BF16)          # ctx^T per pair [d, q]

            with tc.tile_pool(name="ps", bufs=1, space="PSUM") as ps:
              with tc.tile_pool(name="px", bufs=1) as px:
                x_s = px.tile([128, 8, S], BF16)
                wv_s = px.tile([128, 8, 512], BF16)
                wqk0_t = px.tile([128, 8, 256], BF16, tag="wqk", bufs=2)
                for kt in range(8):
                    eng = nc.gpsimd if kt % 2 == 0 else nc.sync
                    eng.dma_start(
                        out=wqk0_t[:, kt, :],
                        in_=xw_d[kt * 128:(kt + 1) * 128, 0:256])
                # ic-major, alternating queues: halves the per-queue DMA
                # issue serialization on the warm-up critical path
                for ic in range(4):
                    for kt in range(8):
                        eng = nc.sync if kt % 2 == 0 else nc.gpsimd
                        eng.dma_start(
                            out=x_s[:, kt, ic * 512:(ic + 1) * 512],
                            in_=xw_d[kt * 128:(kt + 1) * 128,
                                     1024 + ic * 512:1024 + (ic + 1) * 512])
                # after x on the sync queue: keeps the shared DMA engines
                # clear for the critical-path x load (wv/v1 needed ~30us in)
                for kt in range(8):
                    nc.sync.dma_start(
                        out=wv_s[:, kt, :],
                        in_=xw_d[kt * 128:(kt + 1) * 128, 3072:3584])
                for jt in range(16):
                    nc.sync.dma_start(
                        out=v1_s[:, jt, :],
                        in_=cons_d[0:1, VPAD_OFF:VPAD_OFF + 520]
                        .to_broadcast([128, 520]))
                bv_s = px.tile([128, 512], BF16)
                nc.sync.dma_start(
                    out=bv_s,
                    in_=cons_d[0:1, BV_OFF:BV_OFF + 512].to_broadcast([128, 512]))
                nc.sync.dma_start(
                    out=bout_s,
                    in_=bout_d[:, :].rearrange("(m p) o -> p m o", p=128))

                def qk_proj_part(p, wqk_t, chunks):
                    for src, ic in chunks:      # src: 1=k, 0=q; ic: 512 tok
                        pj = ps.tile([128, 512], F32, tag="w", bufs=2)
                        for kt in range(8):
                            nc.tensor.matmul(
                                out=pj,
                                lhsT=wqk_t[:, kt, src * 128:(src + 1) * 128],
                                rhs=x_s[:, kt, ic * 512:(ic + 1) * 512],
                                start=(kt == 0), stop=(kt == 7))
                        nc.vector.tensor_scalar_add(
                            qk_s[:, p % 2, src, ic * 512:(ic + 1) * 512],
                            pj, bqk_s[:, 4 * src + p, 0:1])

                def qk_proj(p, wqk_t):
                    # k first (scores need all of k, only half of q at a time)
                    qk_proj_part(p, wqk_t,
                                 [(1, ic) for ic in range(4)]
                                 + [(0, ic) for ic in range(4)])

                def v_proj(g, jts):
                    # one head-pair's v columns (g = pair index), so the work
                    # spreads across the schedule instead of lumping 34us
                    for jt in jts:
                        pj = ps.tile([128, 512], F32, tag="w", bufs=2)
                        for kt in range(8):
                            nc.tensor.matmul(
                                out=pj[:, 0:128],
                                lhsT=x_s[:, kt, jt * 128:(jt + 1) * 128],
                                rhs=wv_s[:, kt, g * 128:(g + 1) * 128],
                                start=(kt == 0), stop=(kt == 7))
                        nc.vector.tensor_add(
                            v1_s[:, jt, g * 130:(g + 1) * 130]
                            .rearrange("p (h c) -> p h c", c=65)[:, :, 0:64],
                            pj[:, 0:128].rearrange("p (h c) -> p h c", c=64),
                            bv_s[:, g * 128:(g + 1) * 128]
                            .rearrange("p (h c) -> p h c", c=64))

                def scores_exp(h, qh, i2s=(0, 1), jtgs=tuple(range(6))):
                    # s-tiles pack 3 key-tiles x 512 queries: 1536-wide exps
                    # amortize ACT access overhead; i2-granular so PV can
                    # chase by 512-query sub-halves
                    p, hl = h // 2, h % 2
                    part = slice(hl * 64, hl * 64 + 64)
                    eb = e_s[(2 * h + qh) % 3]
                    for i2 in i2s:
                        q0 = qh * 1024 + i2 * 512
                        for jtg in jtgs:           # jt triples, last has 1
                            j0, j1 = 3 * jtg, min(3 * jtg + 3, 16)
                            st = ps.tile([128, 3, 512], F32, tag="s", bufs=2)
                            for j in range(j0, j1):
                                nc.tensor.matmul(
                                    out=st[:, j - j0, :],
                                    lhsT=qk_s[part, p % 2, 1, j * 128:(j + 1) * 128],
                                    rhs=qk_s[part, p % 2, 0, q0:q0 + 512],
                                    start=True, stop=True)
                            nc.scalar.activation(
                                out=eb[:, j0:j1, i2 * 512:(i2 + 1) * 512],
                                in_=st[:, 0:j1 - j0, :],
                                func=EXP, scale=SCALE)

                def pv_part(h, qh, qbs):
                    # two query-blocks share one psum slot; one batched
                    # reciprocal per pair keeps DVE ahead of the PE
                    p, hl = h // 2, h % 2
                    eb = e_s[(2 * h + qh) % 3]
                    qbs = list(qbs)
                    for qq in range(0, len(qbs), 2):
                        pv = ps.tile([128, 2, 256], F32, tag="w", bufs=2)
                        for j2, qb in enumerate(qbs[qq:qq + 2]):
                            for jt in range(16):
                                nc.tensor.matmul(
                                    out=pv[:, j2, 0:65],
                                    lhsT=eb[:, jt, qb * 128:(qb + 1) * 128],
                                    rhs=v1_s[:, jt, h * 65:h * 65 + 65],
                                    start=(jt == 0), stop=(jt == 15))
                        rcp = pa.tile([128, 2], F32, tag="rcp", bufs=4)
                        nc.vector.reciprocal(rcp, pv[:, :, 64])
                        for j2, qb in enumerate(qbs[qq:qq + 2]):
                            nc.vector.tensor_scalar_mul(
                                ctq_s[:, p % 2, qh * 8 + qb, hl * 64:hl * 64 + 64],
                                pv[:, j2, 0:64], rcp[:, j2:j2 + 1])

                def pv_half(h, qh):
                    pv_part(h, qh, range(8))

                def transposes(p, qbs):
                    for qb in qbs:
                        nc.sync.dma_start_transpose(
                            out=ctx_s[:, p, qb * 128:(qb + 1) * 128],
                            in_=ctq_s[:, p % 2, qb, :])

                def wqk_load(p):
                    t = px.tile([128, 8, 256], BF16, tag="wqk", bufs=2)
                    for kt in range(8):
                        nc.sync.dma_start(
                            out=t[:, kt, :],
                            in_=xw_d[kt * 128:(kt + 1) * 128,
                                     p * 256:(p + 1) * 256])
                    return t

                wz = px.tile([1, 640], BF16)
                nc.vector.memset(wz, 0.5)

                def pe_warm(n):
                    # keep the PE busy (and its p-state hot) while x loads
                    for _ in range(n):
                        st = ps.tile([128, 3, 512], F32, tag="s", bufs=2)
                        nc.tensor.matmul(
                            out=st[:, 0, :],
                            lhsT=wz[0:1, 0:128], rhs=wz[0:1, 0:512],
                            start=True, stop=True)

                # ---- warm-up: pair-0 qk (k first, then q by halves so the
                # first scores start as early as possible); pair-0's v columns
                # fill the PE while ACT chews head-0 exps.
                # interleave pair-0 qk chunks with the first score triples:
                # triple g only needs k for tokens 384g..384g+384, so the
                # first exp fires ~10us in instead of ~20
                pe_warm(12)
                qk_proj_part(0, wqk0_t, [(1, 0), (0, 0), (0, 1)])
                scores_exp(0, 0, jtgs=(0,))
                qk_proj_part(0, wqk0_t, [(1, 1)])
                scores_exp(0, 0, jtgs=(1,))
                qk_proj_part(0, wqk0_t, [(1, 2)])
                scores_exp(0, 0, jtgs=(2, 3))
                qk_proj_part(0, wqk0_t, [(1, 3)])
                scores_exp(0, 0, jtgs=(4, 5))
                qk_proj_part(0, wqk0_t, [(0, 2), (0, 3)])
                wqk_t = wqk_load(1)
                scores_exp(0, 1)
                v_proj(0, range(16))

                # ---- steady state half-steps: pv of step k-2 (deps ready),
                # scores of step k (keeps ACT fed), then filler PE work packed
                # as late as its deadline allows (late heads have no fillers
                # left, so deadline-packing minimizes end-game PE idle).
                for k in range(2, 14):
                    h2, qh2 = (k - 2) // 2, (k - 2) % 2
                    h, qh = k // 2, k % 2
                    pv_half(h2, qh2)
                    scores_exp(h, qh)
                    q = k // 4 + 1                 # pair whose qk is due
                    g = k // 4                     # pair whose v cols are due
                    if k % 4 == 2 and q <= 3:
                        qk_proj_part(q, wqk_t, [(1, ic) for ic in range(4)])
                    elif k % 4 == 3 and q <= 3:
                        qk_proj_part(q, wqk_t, [(0, ic) for ic in range(4)])
                    elif k % 4 == 0:
                        if g <= 3:
                            v_proj(g, range(8))
                        if k == 4:
                            wqk_t = wqk_load(2)
                        if k == 8:
                            wqk_t = wqk_load(3)
                    elif k % 4 == 1:
                        if g <= 3:
                            v_proj(g, range(8, 16))
                        if k > 4:                  # pair k//4-1 ctx done
                            transposes(k // 4 - 1, range(16))
                pv_half(6, 0)
                scores_exp(7, 0)
                pv_half(6, 1)

              # px closed: x/wv/wqk SBUF freed for wo/ot
              with tc.tile_pool(name="pl", bufs=1) as pl:
                wo_s = pl.tile([128, 4, E], BF16)
                for ct in range(4):
                    nc.gpsimd.dma_start(
                        out=wo_s[:, ct, :],
                        in_=wo_d[ct * 128:(ct + 1) * 128, :])

                def out_proj(i2):
                    for et in range(8):
                        po = ps.tile([128, 512], F32, tag="w", bufs=2)
                        for ct in range(4):
                            nc.tensor.matmul(
                                out=po,
                                lhsT=wo_s[:, ct, et * 128:(et + 1) * 128],
                                rhs=ctx_s[:, ct, i2 * 512:(i2 + 1) * 512],
                                start=(ct == 0), stop=(ct == 3))
                        ot = pl.tile([128, 512], F32, tag="ot", bufs=4)
                        nc.vector.tensor_scalar_add(ot, po, bout_s[:, et, 0:1])
                        nc.sync.dma_start(
                            out=out_d[et * 128:(et + 1) * 128,
                                      i2 * 512:(i2 + 1) * 512],
                            in_=ot)

                # tail: weave the last scores half with the pv/out-proj
                # chase at sub-half granularity
                scores_exp(7, 1, i2s=(0,))
                pv_part(7, 0, range(4))
                transposes(3, range(4))
                out_proj(0)
                scores_exp(7, 1, i2s=(1,))
                pv_part(7, 0, range(4, 8))
                transposes(3, range(4, 8))
                out_proj(1)
                pv_part(7, 1, range(4))
                transposes(3, range(8, 12))
                out_proj(2)
                pv_part(7, 1, range(4, 8))
                transposes(3, range(12, 16))
                out_proj(3)
    nc.compile()
    return nc


_NC = None


def _get_nc():
    global _NC
    if _NC is None:
        _NC = build_nc()
    return _NC


def make_in_maps(query, Wqkv, bqkv, Wout, bout):
    query = np.asarray(query, dtype=np.float32)
    Wqkv = np.asarray(Wqkv, dtype=np.float32)
    bqkv = np.asarray(bqkv, dtype=np.float32)
    Wout = np.asarray(Wout, dtype=np.float32)
    bout = np.asarray(bout, dtype=np.float32)
    bf = ml_dtypes.bfloat16

    in_maps = []
    for c in range(8):
        b, hh = c // 2, c % 2
        heads = np.arange(hh * HL, hh * HL + HL)
        dims = (heads[:, None] * HD + np.arange(HD)[None, :]).reshape(-1)  # [512]
        q_rows, k_rows, v_rows = dims, E + dims, 2 * E + dims

        xw = np.empty((E, 3584), bf)
        for p in range(4):
            pd = dims[p * 128:(p + 1) * 128]
            xw[:, p * 256:p * 256 + 128] = Wqkv[pd].T.astype(bf)
            xw[:, p * 256 + 128:p * 256 + 256] = Wqkv[E + pd].T.astype(bf)
        xw[:, 1024:3072] = query[b].T.astype(bf)
        xw[:, 3072:3584] = Wqkv[v_rows].T.astype(bf)

        # bqk rows: [q pair0..3 | k pair0..3], each pair-major 128 rows
        bqk = np.concatenate([bqkv[q_rows], bqkv[k_rows]]).reshape(E, 1)

        cons = np.zeros((1, CONS_LEN), bf)
        cons[0, ONES_OFF:ONES_OFF + 128] = 1.0
        cons[0, BV_OFF:BV_OFF + 512] = bqkv[v_rows].astype(bf)
        vpad = np.zeros(520, bf)
        for i in range(8):
            vpad[i * 65 + 64] = 1.0
        cons[0, VPAD_OFF:VPAD_OFF + 520] = vpad

        wo = np.ascontiguousarray(Wout[:, dims].T).astype(bf)   # [512, E]
        bo = (bout if hh == 0 else np.zeros_like(bout)).reshape(E, 1)

        in_maps.append({
            "xw": xw, "bqk": np.ascontiguousarray(bqk),
            "cons": cons, "wo": wo, "bout": np.ascontiguousarray(bo),
        })
    return in_maps


def gather(results):
    out = np.empty((B, S, E), np.float32)
    for b in range(B):
        acc = results[2 * b]["outT"] + results[2 * b + 1]["outT"]   # [E, S]
        out[b] = acc.T
    return out


def kernel(query, key, value, Wqkv, bqkv, Wout, bout):
    # key/value are unused by the reference module (qkv all from query)
    nc = _get_nc()
    in_maps = make_in_maps(query, Wqkv, bqkv, Wout, bout)
    res = run_bass_kernel_spmd(nc, in_maps, list(range(8)))
    return gather(res.results)
